# revision 53
# baseline (speedup 1.0000x reference)
"""Bass/Trainium2 SPMD kernel for nn_BlockCausalTransformer_884763263652.

Sharding over 8 NeuronCores (one chip):
  - Residual x sequence-sharded: core c owns rows [c*256, (c+1)*256).
  - Attention head-sharded: core c owns heads (2c, 2c+1); q/k/v/mix built
    from a column shard of w_qkv (+ w_mix), attention runs fully local.
  - FFN sequence-sharded: each core runs the whole FFN on its own rows.
  - Communication per layer: AllGather of rmsnorm rows (fp8 h^T), and
    AllToAll of o^T which hands every core the [contraction, its-rows]
    slice for the output projection.
  - Attention: paired [128,1024] sim tiles (one exp per pair), AV batched
    into [65,512] accumulators with column-suffix matmuls on the
    block-diagonal; softmax division by the appended ones-row denominator
    via reciprocal_approx_fast and a ones-broadcast matmul.
  - All dense projections (qkv/mix, attn-out, ff_in, ff_out) run in fp8
    DoubleRow perf mode: weights are quantized to fp8e4 host-side with a
    x16 scale (descaled at PSUM evacuation) and pairs of 128-contraction
    chunks execute as one K=256 matmul, halving PE streaming time.
RMSNorm weights are folded host-side into the following matmul weights;
final_norm_w is applied host-side to the gathered output.
"""

import sys

sys.path.insert(0, "/opt/trn_rl_repo")

import numpy as np
import ml_dtypes

import concourse.bacc as bacc
import concourse.tile as tile
import concourse.mybir as mybir
from concourse.bass_utils import run_bass_kernel_spmd

F32 = mybir.dt.float32
BF16 = mybir.dt.bfloat16
F8 = mybir.dt.float8e4
AF = mybir.ActivationFunctionType
DR = mybir.MatmulPerfMode.DoubleRow
MUL = mybir.AluOpType.mult
ADD = mybir.AluOpType.add

DEPTH = 4
DIM = 1024
HEADS = 16
DIM_HEAD = 64
FF_HID = 2730
N = 2048
EPS = 1.1920929e-07
SCALE = DIM_HEAD ** -0.5
N_CORES = 8

ROWS = N // N_CORES            # 256 sequence rows per core
H_PER = HEADS // N_CORES       # 2 heads per core
KD = DIM // 128                # 8 feature chunks
KP = KD // 2                   # 4 feature chunk-pairs (DoubleRow)
KF = (FF_HID + 127) // 128     # 22 hidden chunks (last ragged: 42 rows)
KFP = KF // 2                  # 11 hidden chunk-pairs
MI_IN = 2 * KF                 # 44 ff_in M-chunks: a chunks then g chunks
NJ = N // 512                  # 4 sequence 512-chunks
QC = N // 128                  # 16 kpos/qpos 128-chunks
W_QKVM = 3 * H_PER * DIM_HEAD + H_PER  # 386 cols: q|k|v (128 each) + mix (2)
W_QKVM_P = 400                 # padded so the k-pair stride is 16B-aligned
FF_ROWS = [min(128, FF_HID - k * 128) for k in range(KF)]

WS = 16.0                      # fp8 weight quantization scale
INV = 1.0 / WS
INV2 = 1.0 / (WS * WS)

_BF = ml_dtypes.bfloat16
_F8 = ml_dtypes.float8_e4m3fn


def _bf16(a):
    return np.ascontiguousarray(a.astype(_BF))


def _f8(a):
    return np.ascontiguousarray(np.clip(np.asarray(a, np.float32),
                                        -240.0, 240.0).astype(_F8))


def _build_bass(n_devices=N_CORES):
    nc = bacc.Bacc("TRN2", target_bir_lowering=False, debug=False,
                   num_devices=n_devices)

    # ---- I/O ----
    x_in = nc.dram_tensor("x", [ROWS, DIM], F32, kind="ExternalInput").ap()
    out_ext = nc.dram_tensor("out", [ROWS, DIM], F32, kind="ExternalOutput").ap()
    wqkvm = nc.dram_tensor("wqkvm", [DEPTH, 128, KD, W_QKVM_P], F8,
                           kind="ExternalInput").ap()
    bmix = nc.dram_tensor("bmix", [DEPTH, H_PER, 1], F32,
                          kind="ExternalInput").ap()
    w_in = nc.dram_tensor("w_in", [DEPTH, MI_IN, 128, KD, 128], BF16,
                          kind="ExternalInput").ap()
    b_in = nc.dram_tensor("b_in", [DEPTH, 128, MI_IN], F32,
                          kind="ExternalInput").ap()
    w_o = nc.dram_tensor("w_o", [DEPTH, 128, KD, DIM], F8,
                         kind="ExternalInput").ap()
    w_ffo = nc.dram_tensor("w_ffo", [DEPTH, 128, KF, DIM], F8,
                           kind="ExternalInput").ap()
    b_ffo = nc.dram_tensor("b_ffo", [DEPTH, 1, DIM], BF16,
                           kind="ExternalInput").ap()
    ropes = nc.dram_tensor("ropes", [2, 128, N], BF16,
                           kind="ExternalInput").ap()
    mt_in = nc.dram_tensor("mt", [128, 128], BF16, kind="ExternalInput").ap()
    id_in = nc.dram_tensor("ident", [128, 128], BF16, kind="ExternalInput").ap()

    # ---- per-layer DRAM bounce buffers for collectives ----
    rg = [list(range(N_CORES))]
    bh_in = [nc.dram_tensor(f"hin{j}", [128, KD * ROWS], F8).ap()
             for j in range(DEPTH)]
    bh_out = [nc.dram_tensor(f"hout{j}", [N_CORES * 128, KD * ROWS], F8,
                             addr_space="Shared").ap()
              for j in range(DEPTH)]
    bo_in = [nc.dram_tensor(f"oin{i}", [N_CORES * 128, ROWS], F8).ap()
             for i in range(DEPTH)]
    bo_out = [nc.dram_tensor(f"oout{i}", [N_CORES * 128, ROWS], F8).ap()
              for i in range(DEPTH)]
    warm_ag_in = nc.dram_tensor("wag_i", [16, 16], BF16).ap()
    warm_ag_out = nc.dram_tensor("wag_o", [N_CORES * 16, 16], BF16,
                                 addr_space="Shared").ap()

    with tile.TileContext(nc) as tc:
        with (
            tc.tile_pool(name="persist", bufs=1) as pp,
            tc.tile_pool(name="wq", bufs=2) as wqp,
            tc.tile_pool(name="wi", bufs=6) as wip,
            tc.tile_pool(name="wstream", bufs=6) as wsp,
            tc.tile_pool(name="hT", bufs=1) as hp,
            tc.tile_pool(name="qk", bufs=1) as qkp,
            tc.tile_pool(name="attn", bufs=8) as ap_,
            tc.tile_pool(name="scratch", bufs=3) as sp,
            tc.tile_pool(name="stage", bufs=1) as stp,
            tc.tile_pool(name="slice", bufs=6) as slp,
            tc.tile_pool(name="mm", bufs=2, space="PSUM") as mmp,
            tc.tile_pool(name="av", bufs=2, space="PSUM") as avp,
        ):
            # persistent tiles
            x_sb = pp.tile([128, 2, DIM], F32, name="x_sb")
            cos_sb = pp.tile([128, N], BF16, name="cos_sb")
            sin_sb = pp.tile([128, N], BF16, name="sin_sb")
            mt = pp.tile([128, 128], BF16, name="mt_sb")
            ident = pp.tile([128, 128], BF16, name="id_sb")
            ones1 = pp.tile([1, 128], BF16, name="ones1")
            fv = pp.tile([128, QC, H_PER, DIM_HEAD], BF16, name="fv")
            # fp8 v (+ ones row for the softmax denominator), padded to 72
            # so the DoubleRow t-pair stride (2*72=144B) is 16B-aligned
            VP = DIM_HEAD + 8
            v_aug = pp.tile([128, QC, H_PER, VP], F8, name="v_aug")
            mix_sb = pp.tile([128, QC, H_PER], BF16, name="mix_sb")
            bmix_sb = pp.tile([H_PER, DEPTH], F32, name="bmix_sb")
            binp_sb = pp.tile([128, DEPTH, MI_IN], F32, name="binp_sb")
            bffo_sb = pp.tile([1, DEPTH, DIM], BF16, name="bffo_sb")
            stat = pp.tile([128, 8], F32, name="stat")
            consts = pp.tile([128, 2], F32, name="consts")
            sq_scr = pp.tile([128, DIM], F32, name="sq_scr")
            nc.vector.memset(consts[:, 0:1], 0.0)
            nc.vector.memset(consts[:, 1:2], EPS)
            nc.const_aps.aps[(F32, 0.0)] = consts[:, 0:1]
            nc.const_aps.aps[(F32, EPS)] = consts[:, 1:2]

            for mi in range(2):
                nc.sync.dma_start(x_sb[:, mi, :], x_in[mi * 128:(mi + 1) * 128, :])
            nc.scalar.dma_start(cos_sb[:], ropes[0])
            nc.scalar.dma_start(sin_sb[:], ropes[1])
            nc.sync.dma_start(mt[:], mt_in[:])
            nc.sync.dma_start(ident[:], id_in[:])
            nc.scalar.dma_start(bmix_sb[:], bmix.rearrange("d h o -> h (d o)"))
            nc.scalar.dma_start(binp_sb[:], b_in.rearrange("d p m -> p d m"))
            nc.scalar.dma_start(bffo_sb[:], b_ffo.rearrange("d o f -> o d f"))
            nc.vector.memset(ones1[:], 1.0)
            nc.vector.memset(v_aug[:, :, :, DIM_HEAD:VP], 0.0)
            nc.vector.memset(v_aug[:, :, :, DIM_HEAD:DIM_HEAD + 1], 1.0)

            def pe_warm(n, tag):
                """Dummy matmuls that keep the PE HAM-warm (and busy)
                while a collective is in flight; each frees instantly."""
                for j in range(n):
                    wt = mmp.tile([128, 512], F32, tag="mm",
                                  name=f"warm_{tag}_{j}")
                    nc.tensor.matmul(wt[:], ident[:], cos_sb[:, 0:512],
                                     start=True, stop=True)

            def rmsnorm_rows(tag):
                """x_sb -> rmsnorm -> bf16 rows [128, 2, DIM] in SBUF."""
                hrow = sp.tile([128, 2, DIM], BF16, tag="hrow",
                               name=f"hrow{tag}")
                for mi in range(2):
                    s = stat[:, 4 * mi:4 * mi + 4]
                    nc.scalar.activation(sq_scr[:], x_sb[:, mi, :], AF.Square,
                                         accum_out=s[:, 0:1])
                    nc.scalar.activation(s[:, 1:2], s[:, 0:1], AF.Sqrt,
                                         bias=EPS, scale=1.0 / DIM)
                    nc.vector.reciprocal(s[:, 2:3], s[:, 1:2])
                    nc.vector.tensor_scalar_mul(hrow[:, mi, :], x_sb[:, mi, :],
                                                s[:, 2:3])
                return hrow

            for i in range(DEPTH):
                # ---------------- attention ----------------
                hrow = rmsnorm_rows(f"a{i}")
                # own h^T shard [128, k, r] via PE transposes, then one
                # contiguous bounce write (2 KB per partition row)
                hT_st = stp.tile([128, KD, ROWS], F8, tag="hT_st",
                                 name=f"hTst{i}")
                for k in range(KD):
                    for mi in range(2):
                        tp = mmp.tile([128, 128], BF16, tag="mm",
                                      name=f"hTt{i}_{k}_{mi}")
                        nc.tensor.transpose(
                            tp[:], hrow[:, mi, k * 128:(k + 1) * 128],
                            ident[:])
                        nc.vector.tensor_copy(
                            hT_st[:, k, mi * 128:(mi + 1) * 128], tp[:])
                nc.sync.dma_start(
                    bh_in[i].rearrange("p (k r) -> p k r", k=KD), hT_st[:])
                nc.gpsimd.collective_compute(
                    "AllGather", mybir.AluOpType.bypass, replica_groups=rg,
                    ins=[bh_in[i][:].opt()], outs=[bh_out[i][:].opt()])
                # fill the AllGather wait with PE-warming work (the first
                # collective additionally sits behind the ~33us CC barrier)
                pe_warm(150 if i == 0 else 56, f"ag{i}")

                # gathered h^T -> SBUF [128, c, k, r] (c-major: each per-c
                # landing DMA writes one contiguous 2KB run per partition);
                # three queues so the landing tail is short.
                hT = hp.tile([128, N_CORES, KD, ROWS], F8, tag="hT")
                src4 = bh_out[i].rearrange("(c p) (k r) -> p c k r",
                                           c=N_CORES, k=KD)
                for c in range(N_CORES):
                    eng = (nc.sync, nc.scalar, nc.gpsimd)[c % 3]
                    eng.dma_start(hT[:, c], src4[:, c])
                # keep the PE warm across the gathered-h landing window too
                pe_warm(64 if i == 0 else 24, f"agl{i}")

                wq_sb = wqp.tile([128, KD, W_QKVM_P], F8, tag="wq")
                nc.sync.dma_start(wq_sb[:], wqkvm[i])
                wo_sb = hp.tile([128, KD, DIM], F8, tag="wo_all")
                nc.sync.dma_start(wo_sb[:], w_o[i])

                def hT_pair(nj, j):
                    # [128, 2(k-pair), 2(core), 256] -> N=512 DoubleRow rhs
                    return hT[:, 2 * nj:2 * nj + 2,
                              2 * j:2 * j + 2, :].rearrange(
                                  "p c k r -> p k c r")

                qT = qkp.tile([128, N], BF16, tag="qT")
                kT = qkp.tile([128, N], BF16, tag="kT")
                vT = qkp.tile([128, N], BF16, tag="vT")
                mixT = qkp.tile([H_PER, N], BF16, tag="mixT")
                outs = [qT, kT, vT]
                for nj in range(NJ):
                    nsl = slice(nj * 512, (nj + 1) * 512)
                    for mi in range(3):
                        ps = mmp.tile([128, 512], F32, tag="mm",
                                      name=f"qkv{i}_{nj}_{mi}")
                        for j in range(KP):
                            nc.tensor.matmul(
                                ps[:],
                                wq_sb[:, 2 * j:2 * j + 2,
                                      mi * 128:(mi + 1) * 128],
                                hT_pair(nj, j), start=(j == 0),
                                stop=(j == KP - 1), perf_mode=DR)
                        if mi == 0:
                            nc.vector.tensor_scalar_mul(outs[0][:, nsl],
                                                        ps[:], SCALE * INV)
                        else:
                            nc.vector.tensor_scalar_mul(outs[mi][:, nsl],
                                                        ps[:], INV)
                if i > 0:
                    for nj in range(NJ):
                        nsl = slice(nj * 512, (nj + 1) * 512)
                        ps = mmp.tile([H_PER, 512], F32, tag="mm",
                                      name=f"mix{i}_{nj}")
                        for j in range(KP):
                            nc.tensor.matmul(
                                ps[:],
                                wq_sb[:, 2 * j:2 * j + 2, 384:384 + H_PER],
                                hT_pair(nj, j), start=(j == 0),
                                stop=(j == KP - 1), perf_mode=DR)
                        nc.scalar.activation(mixT[:, nsl], ps[:], AF.Sigmoid,
                                             bias=bmix_sb[:, i:i + 1],
                                             scale=INV)

                # rope
                qrot = qkp.tile([128, N], BF16, tag="qrot")
                krot = qkp.tile([128, N], BF16, tag="krot")
                for (src, dst) in ((qT, qrot), (kT, krot)):
                    for nj in range(NJ):
                        nsl = slice(nj * 512, (nj + 1) * 512)
                        sw = mmp.tile([128, 512], F32, tag="mm")
                        nc.tensor.matmul(sw[:], mt[:], src[:, nsl],
                                         start=True, stop=True)
                        t1 = sp.tile([128, 512], BF16, tag="ropet1")
                        nc.vector.tensor_mul(t1[:], sw[:], sin_sb[:, nsl])
                        t2 = sp.tile([128, 512], BF16, tag="ropet2")
                        nc.vector.tensor_mul(t2[:], src[:, nsl],
                                             cos_sb[:, nsl])
                        nc.vector.tensor_add(dst[:, nsl], t1[:], t2[:])

                # v^T -> v rows in v_aug (+ fused lerp toward first_v, i>0)
                if i > 0:
                    for t in range(QC):
                        tp = mmp.tile([128, H_PER], BF16, tag="mm",
                                      name=f"mixTt{i}_{t}")
                        nc.tensor.transpose(tp[:],
                                            mixT[:, t * 128:(t + 1) * 128],
                                            ident[0:H_PER, 0:H_PER])
                        nc.vector.tensor_copy(mix_sb[:, t, :], tp[:])
                for t in range(QC):
                    tp = mmp.tile([128, 128], BF16, tag="mm",
                                  name=f"vT{i}_{t}")
                    nc.tensor.transpose(tp[:], vT[:, t * 128:(t + 1) * 128],
                                        ident[:])
                    tpv = tp[:].rearrange("p (h d) -> p h d", h=H_PER)
                    if i == 0:
                        nc.vector.tensor_copy(v_aug[:, t, :, 0:DIM_HEAD], tpv)
                    else:
                        d = sp.tile([128, H_PER, DIM_HEAD], BF16, tag="lerpd")
                        nc.vector.tensor_sub(d[:], fv[:, t, :, :], tpv)
                        for h in range(H_PER):
                            nc.vector.scalar_tensor_tensor(
                                v_aug[:, t, h, 0:DIM_HEAD],
                                d[:, h, :],
                                mix_sb[:, t, h:h + 1],
                                tpv[:, h, :],
                                op0=MUL,
                                op1=ADD)
                if i == 0:
                    nc.vector.tensor_copy(fv[:], v_aug[:, :, :, 0:DIM_HEAD])

                # attention: quadrant-packed sims (2 heads x 2 kpos-halves of
                # one key block run as four concurrent 64x64 PE tiles), fp8
                # attn, DoubleRow AV over t-pairs on the full blocks, and
                # normalization pipelined one q-chunk behind from SBUF copies
                oT_loc = qkp.tile([128, N], F8, tag="oT")
                bo_dst = bo_in[i].rearrange("(j p) r -> p j r", p=128)

                def emit_normalize(h, qj, ow_nm, den):
                    hsl = slice(h * DIM_HEAD, (h + 1) * DIM_HEAD)
                    q0 = qj * 512
                    rcp = sp.tile([1, 512], F32, tag="rcp")
                    nc.vector.reciprocal_approx_fast(rcp[:], den[:])
                    rcpw = sp.tile([1, 512], BF16, tag="rcpw")
                    nc.vector.tensor_copy(rcpw[:], rcp[:])
                    rb = mmp.tile([DIM_HEAD, 512], F32, tag="mm",
                                  name=f"rb{i}_{h}_{qj}")
                    nc.tensor.matmul(rb[:], ones1[0:1, 0:DIM_HEAD],
                                     rcpw[:], start=True, stop=True)
                    rbs = sp.tile([DIM_HEAD, 512], BF16, tag="rbs")
                    nc.vector.tensor_copy(rbs[:], rb[:])
                    nc.vector.tensor_mul(oT_loc[hsl, q0:q0 + 512],
                                         ow_nm[:], rbs[:])
                    if h == H_PER - 1:
                        nc.sync.dma_start(
                            bo_dst[:, 2 * qj:2 * qj + 2, :],
                            oT_loc[:, q0:q0 + 512].rearrange(
                                "p (j r) -> p j r", j=2))

                pend = []
                for qj in range(NJ):
                    q0 = qj * 512
                    o_w = [avp.tile([VP, 512], F32, tag="av",
                                    name=f"ow{i}_{qj}_{h}")
                           for h in range(H_PER)]
                    for pn in pend:
                        emit_normalize(*pn)
                    pend = []

                    def quad_sims(sim, t, off):
                        for h in range(H_PER):
                            for kh in range(2):
                                nc.tensor.matmul(
                                    sim[kh * 64:kh * 64 + 64,
                                        h * 512 + off:(h + 1) * 512],
                                    krot[h * 64:h * 64 + 64,
                                         t * 128 + kh * 64:
                                         t * 128 + kh * 64 + 64],
                                    qrot[h * 64:h * 64 + 64,
                                         q0 + off:q0 + 512],
                                    start=True, stop=True)

                    # AVs are emitted two steps behind their sims/exps so the
                    # PE never head-of-line blocks on the scalar-engine exp
                    av_q = []

                    def drain_av(keep):
                        while len(av_q) > keep:
                            av_q.pop(0)()

                    # full key blocks: t-pairs, one exp per t, DoubleRow AV
                    for pi in range(2 * qj):
                        attn = ap_.tile([128, 2, H_PER, 512], F8, tag="attn",
                                        name=f"at{i}_{qj}_{pi}")
                        for tt in range(2):
                            t = 2 * pi + tt
                            sim = mmp.tile([128, 1024], F32, tag="sim",
                                           name=f"sim{i}_{qj}_{pi}_{tt}")
                            quad_sims(sim, t, 0)
                            nc.scalar.activation(
                                attn[:, tt, :, :].rearrange(
                                    "p h q -> p (h q)"),
                                sim[:, :], AF.Exp)

                        def mk_av(attn=attn, pi=pi):
                            def em():
                                for h in range(H_PER):
                                    nc.tensor.matmul(
                                        o_w[h][:, :],
                                        v_aug[:, 2 * pi:2 * pi + 2, h, :],
                                        attn[:, :, h, :],
                                        start=(pi == 0), stop=False,
                                        perf_mode=DR)
                            return em
                        av_q.append(mk_av())
                        drain_av(2)
                    # block-diagonal: per-t, normal mode, column-suffix AV
                    for dt in range(4):
                        t = 4 * qj + dt
                        off = dt * 128
                        sim = mmp.tile([128, 1024], F32, tag="sim",
                                       name=f"simd{i}_{qj}_{dt}")
                        attn_d = ap_.tile([128, H_PER, 512], F8, tag="attnd",
                                          name=f"atd{i}_{qj}_{dt}")
                        quad_sims(sim, t, off)
                        for h in range(H_PER):
                            nc.scalar.activation(
                                attn_d[:, h, off:512],
                                sim[:, h * 512 + off:(h + 1) * 512], AF.Exp)

                        def mk_avd(attn_d=attn_d, t=t, dt=dt, off=off):
                            def em():
                                for h in range(H_PER):
                                    nc.tensor.matmul(
                                        o_w[h][:, off:512],
                                        v_aug[:, t, h, :],
                                        attn_d[:, h, off:512],
                                        start=(qj == 0 and dt == 0),
                                        stop=(dt == 3))
                            return em
                        av_q.append(mk_avd())
                        drain_av(2)
                    drain_av(0)
                    for h in range(H_PER):
                        ow_nm = sp.tile([DIM_HEAD, 512], BF16, tag="ownm",
                                        name=f"ownm{i}_{qj}_{h}")
                        den = sp.tile([1, 512], F32, tag="den",
                                      name=f"den{i}_{qj}_{h}")
                        nc.vector.tensor_copy(ow_nm[:], o_w[h][0:DIM_HEAD, :])
                        nc.vector.tensor_copy(
                            den[:], o_w[h][DIM_HEAD:DIM_HEAD + 1, :])
                        pend.append((h, qj, ow_nm, den))
                for pn in pend:
                    emit_normalize(*pn)
                nc.gpsimd.collective_compute(
                    "AllToAll", mybir.AluOpType.bypass, replica_groups=rg,
                    ins=[bo_in[i][:].opt()], outs=[bo_out[i][:].opt()])
                pe_warm(36, f"a2a{i}")

                # delta_attn = o_full^T[:, rows_c].T @ W_o  (+= into x_sb);
                # prefetch all o slices, warm across the landing window
                osls = []
                for j in range(KP):
                    osl = slp.tile([128, 2, 2, 128], F8, tag="osl",
                                   name=f"osl{i}_{j}")
                    eng = nc.sync if j % 2 == 0 else nc.scalar
                    eng.dma_start(
                        osl[:],
                        bo_out[i][j * 256:(j + 1) * 256, :].rearrange(
                            "(a p) (m q) -> p a m q", a=2, m=2))
                    osls.append(osl)
                pe_warm(16, f"a2al{i}")
                dps = [[(avp if _a == 0 else mmp).tile(
                    [128, 512], F32, tag=("av" if _a == 0 else "sim"),
                    name=f"dps{i}_{_a}_{_b}")
                    for _b in range(2)] for _a in range(2)]
                for j in range(KP):
                    osl = osls[j]
                    for mi in range(2):
                        for nj2 in range(2):
                            nc.tensor.matmul(
                                dps[mi][nj2][:], osl[:, :, mi, :],
                                wo_sb[:, 2 * j:2 * j + 2,
                                      nj2 * 512:(nj2 + 1) * 512],
                                start=(j == 0), stop=(j == KP - 1),
                                perf_mode=DR)
                for mi in range(2):
                    for nj2 in range(2):
                        nsl = slice(nj2 * 512, (nj2 + 1) * 512)
                        nc.vector.scalar_tensor_tensor(
                            x_sb[:, mi, nsl], dps[mi][nj2][:], INV,
                            x_sb[:, mi, nsl], op0=MUL, op1=ADD)

                # ------------- feedforward (sequence-parallel) -------------
                hfrow = rmsnorm_rows(f"f{i}")
                hfT = stp.tile([128, KD, 2 * 128], BF16, tag="hfT",
                               name=f"hfT{i}")
                for k in range(KD):
                    for mi in range(2):
                        tp = mmp.tile([128, 128], BF16, tag="mm",
                                      name=f"hfTt{i}_{k}_{mi}")
                        nc.tensor.transpose(
                            tp[:], hfrow[:, mi, k * 128:(k + 1) * 128],
                            ident[:])
                        nc.vector.tensor_copy(
                            hfT[:, k, mi * 128:(mi + 1) * 128], tp[:])

                # a^T / g^T / act^T chunks over full hidden (own rows only)
                act_sb = qkp.tile([128, KF, 2 * 128], BF16, tag="act")
                for kc in range(KF):
                    rows = FF_ROWS[kc]
                    g_c = sp.tile([128, 2 * 128], BF16, tag="ffg",
                                  name=f"ffg{i}_{kc}")
                    for part in range(2):  # 0 = a, 1 = g
                        ci = part * KF + kc
                        wmi = wip.tile([128, KD, 128], BF16, tag="wi",
                                       name=f"wi{i}_{ci}")
                        nc.sync.dma_start(wmi[:], w_in[i, ci])
                        # a-chunks and g-chunks on separate PSUM tag slots so
                        # the next chunk's matmuls never wait on an evacuation
                        ps = mmp.tile([128, 2 * 128], F32,
                                      tag=("sim" if part == 0 else "mm"),
                                      name=f"ffps{i}_{ci}")
                        for k in range(KD):
                            nc.tensor.matmul(
                                ps[0:rows, :], wmi[:, k, 0:rows],
                                hfT[:, k, :], start=(k == 0),
                                stop=(k == KD - 1))
                        bias = binp_sb[0:rows, i, ci:ci + 1]
                        if part == 0:
                            nc.vector.tensor_scalar_add(
                                act_sb[0:rows, kc, :], ps[0:rows, :], bias)
                        else:
                            nc.scalar.activation(g_c[0:rows, :],
                                                 ps[0:rows, :], AF.Gelu,
                                                 bias=bias)
                    nc.vector.tensor_mul(act_sb[0:rows, kc, :],
                                         act_sb[0:rows, kc, :],
                                         g_c[0:rows, :])

                # delta_ff = act^T.T @ W_ffo + b_ffo  (+= into x_sb)
                fps = [[(avp if _a == 0 else mmp).tile(
                    [128, 512], F32, tag=("av" if _a == 0 else "sim"),
                    name=f"fps{i}_{_a}_{_b}")
                    for _b in range(2)] for _a in range(2)]
                # w_ffo fetched two hidden chunks per DMA (contiguous per
                # partition) so the chunk supply outruns the matmul stream
                # fp8 w_ffo (x16 scale, bf16 act, normal-mode mixed matmul)
                # halves the chunk supply bytes that gated this phase
                for m in range(KFP):
                    wsl = wsp.tile([128, 2, DIM], F8, tag="wf",
                                   name=f"wf{i}_{m}")
                    nc.sync.dma_start(wsl[:], w_ffo[i, :, 2 * m:2 * m + 2, :])
                    for kk in range(2):
                        kc = 2 * m + kk
                        rows = FF_ROWS[kc]
                        for mi in range(2):
                            for nj2 in range(2):
                                nc.tensor.matmul(
                                    fps[mi][nj2][:],
                                    act_sb[0:rows, kc,
                                           mi * 128:(mi + 1) * 128],
                                    wsl[0:rows, kk,
                                        nj2 * 512:(nj2 + 1) * 512],
                                    start=(kc == 0), stop=False)
                for mi in range(2):
                    for nj2 in range(2):
                        nsl = slice(nj2 * 512, (nj2 + 1) * 512)
                        nc.tensor.matmul(fps[mi][nj2][:], ones1[:, 0:128],
                                         bffo_sb[:, i, nsl], start=False,
                                         stop=True)
                        nc.vector.scalar_tensor_tensor(
                            x_sb[:, mi, nsl], fps[mi][nj2][:], INV,
                            x_sb[:, mi, nsl], op0=MUL, op1=ADD)

            # ---------------- final rmsnorm ----------------
            for mi in range(2):
                s = stat[:, 4 * mi:4 * mi + 4]
                nc.scalar.activation(sq_scr[:], x_sb[:, mi, :], AF.Square,
                                     accum_out=s[:, 0:1])
                nc.scalar.activation(s[:, 1:2], s[:, 0:1], AF.Sqrt,
                                     bias=EPS, scale=1.0 / DIM)
                nc.vector.reciprocal(s[:, 2:3], s[:, 1:2])
                nc.vector.tensor_scalar_mul(sq_scr[:], x_sb[:, mi, :],
                                            s[:, 2:3])
                nc.sync.dma_start(out_ext[mi * 128:(mi + 1) * 128, :],
                                  sq_scr[:])

    nc.compile()
    return nc


_NC_CACHE = {}


def _get_nc():
    if "nc" not in _NC_CACHE:
        _NC_CACHE["nc"] = _build_bass()
    return _NC_CACHE["nc"]


def _rope_tables():
    freqs = 1.0 / (10000.0 ** (np.arange(0, DIM_HEAD, 2, dtype=np.float64)
                               / DIM_HEAD))
    ang = np.arange(N, dtype=np.float64)[:, None] * freqs[None, :]
    cos = np.repeat(np.cos(ang), 2, axis=-1).T  # [64, N]
    sin = np.repeat(np.sin(ang), 2, axis=-1).T
    cos2 = np.concatenate([cos, cos], axis=0)   # [128, N] two heads
    sin2 = np.concatenate([sin, sin], axis=0)
    return np.stack([cos2, sin2]).astype(np.float32)


def _swap_matrix():
    # lhsT for qswap^T = M @ q^T with M[2i,2i+1] = -1, M[2i+1,2i] = +1
    m = np.zeros((DIM_HEAD, DIM_HEAD), np.float32)
    for j in range(DIM_HEAD // 2):
        m[2 * j, 2 * j + 1] = 1.0
        m[2 * j + 1, 2 * j] = -1.0
    mt = np.zeros((128, 128), np.float32)
    mt[0:64, 0:64] = m
    mt[64:128, 64:128] = m
    return mt


def _make_in_maps(tokens, attn_norm_w, w_qkv, w_mix, b_mix, ff_norm_w,
                  w_ff_in, b_ff_in, w_attn_out, w_ff_out, b_ff_out):
    ropes = _bf16(_rope_tables())
    mt = _bf16(_swap_matrix())
    ident = _bf16(np.eye(128, dtype=np.float32))

    # shared (identical on every core) tensors; fp8 weights carry a x16
    # scale that the kernel removes at PSUM evacuation
    wo_l, wf_l, bf_l = [], [], []
    for i in range(DEPTH):
        wo_l.append(w_attn_out[i].reshape(KD, 128, DIM).transpose(1, 0, 2))
        Wf = np.zeros((KF * 128, DIM), np.float32)
        Wf[:FF_HID, :] = w_ff_out[i]
        wf_l.append(Wf.reshape(KF, 128, DIM).transpose(1, 0, 2))
        bf_l.append(b_ff_out[i].reshape(1, DIM))
    w_o_np = _f8(np.stack(wo_l) * WS)
    w_ffo_np = _f8(np.stack(wf_l) * WS)
    b_ffo_np = _bf16(np.stack(bf_l) * WS)

    wi_l, bi_l = [], []
    for i in range(DEPTH):
        W = w_ff_in[i] * ff_norm_w[i][:, None]        # [1024, 5460]
        chunks = []
        bp = np.zeros((128, MI_IN), np.float32)
        for part in range(2):
            for kc in range(KF):
                ci = part * KF + kc
                rows = FF_ROWS[kc]
                blk = np.zeros((DIM, 128), np.float32)
                c0 = part * FF_HID + kc * 128
                blk[:, :rows] = W[:, c0:c0 + rows]
                chunks.append(blk.reshape(KD, 128, 128).transpose(1, 0, 2))
                bp[:rows, ci] = b_ff_in[i, c0:c0 + rows]
        wi_l.append(np.stack(chunks))                 # [MI_IN, 128, KD, 128]
        bi_l.append(bp)
    w_in_np = _bf16(np.stack(wi_l))
    b_in_np = np.ascontiguousarray(np.stack(bi_l), dtype=np.float32)

    in_maps = []
    for c in range(N_CORES):
        m = {}
        m["x"] = np.ascontiguousarray(
            tokens[0, c * ROWS:(c + 1) * ROWS, :]).astype(np.float32)
        wq_l, bm_l = [], []
        for i in range(DEPTH):
            W = w_qkv[i] * attn_norm_w[i][:, None]
            Wm = w_mix[i] * attn_norm_w[i][:, None]
            cols = []
            for part in range(3):  # q, k, v
                for h in (2 * c, 2 * c + 1):
                    base = part * HEADS * DIM_HEAD + h * DIM_HEAD
                    cols.append(W[:, base:base + DIM_HEAD])
            cols.append(Wm[:, 2 * c:2 * c + 2])
            cols.append(np.zeros((DIM, W_QKVM_P - W_QKVM), np.float32))
            Wc = np.concatenate(cols, axis=1)             # [1024, 400]
            wq_l.append(Wc.reshape(KD, 128, W_QKVM_P).transpose(1, 0, 2))
            bm_l.append(b_mix[i, 2 * c:2 * c + 2].reshape(H_PER, 1))
        m["wqkvm"] = _f8(np.stack(wq_l) * WS)
        m["bmix"] = np.ascontiguousarray(np.stack(bm_l), dtype=np.float32)

        m["w_in"] = w_in_np
        m["b_in"] = b_in_np
        m["w_o"] = w_o_np
        m["w_ffo"] = w_ffo_np
        m["b_ffo"] = b_ffo_np
        m["ropes"] = ropes
        m["mt"] = mt
        m["ident"] = ident
        in_maps.append(m)
    return in_maps


def kernel(tokens, attn_norm_w, w_qkv, w_attn_out, w_mix, b_mix,
           ff_norm_w, w_ff_in, b_ff_in, w_ff_out, b_ff_out, final_norm_w,
           _trace=False):
    tokens = np.asarray(tokens, dtype=np.float32)
    nc = _get_nc()
    in_maps = _make_in_maps(
        tokens,
        np.asarray(attn_norm_w, np.float32), np.asarray(w_qkv, np.float32),
        np.asarray(w_mix, np.float32), np.asarray(b_mix, np.float32),
        np.asarray(ff_norm_w, np.float32), np.asarray(w_ff_in, np.float32),
        np.asarray(b_ff_in, np.float32), np.asarray(w_attn_out, np.float32),
        np.asarray(w_ff_out, np.float32), np.asarray(b_ff_out, np.float32))
    res = run_bass_kernel_spmd(nc, in_maps, core_ids=list(range(N_CORES)),
                               trace=_trace)
    out = np.concatenate([res.results[c]["out"] for c in range(N_CORES)],
                         axis=0)
    out = out * np.asarray(final_norm_w, np.float32)[None, :]
    kernel.last_results = res
    return out.reshape(1, N, DIM).astype(np.float32)
